# revision 41
# baseline (speedup 1.0000x reference)
"""Trainium2 Bass kernel for AnisotropicGaussianSampler.

Reference computation (H=W=128, N=4096 samples, B=8):
    corr[b,n] = (1/(H*W)) * sum_{h,w} A[b,h,w] * exp(-eh[h,n]) * exp(-ew[w,n])
    eh[h,n] = (h/H - mu[n,0])^2 / (2*sigma[n,0]^2)   (separable in h and w)

Factorization used on-device (per sample column n):
    Ph[h,n] = exp(-0.5 * zh^2),  zh = (mu0[n] - h/H) / sigma0[n]
    Pw[w,n] = exp(-0.5 * zw^2)
    N_b[w,n] = sum_h A[b,h,w] * Ph[h,n]          (matmul, lhsT = A_b as stored)
    corr[b,n] = (1/(H*W)) * sum_w Pw[w,n]*N_b[w,n]  (mul + ones-reduce matmul)

Precision split: the z tables are produced in float32r (single-pass fp32
matmul; z is cancellation-sensitive), while the big batch matmuls run in
float16 (same 1 cycle/row as f32r but ~10x faster weight loads via FWL;
fp16's 11-bit mantissa keeps the result within ~2e-3).

Table prep: 1/sigma and mu/sigma are computed across 128 partitions (fast DVE)
in a [128, (q,t,c)] column tile, PE-transposed to [16, 128] in one shot, copied
to SBUF (rounding to f32r), and DMA-gathered into one [2, 512] row tile per
axis. A single K=2 matmul per axis (constant lhsT rows {ones, -grid}) then
produces z in PSUM; ACT squares and exponentiates it.

DMA routing: all small loads are packed into ONE [128, 144] bundle (mu, sigma,
identity) on the sync HWDGE ring; zconst/onehots ride the scalar ring; the
512KB activations load is a single casting DMA (f32 -> f16) on gpsimd SWDGE.

The batch loop is software-pipelined (skew 2) so the DVE multiply of batch b
overlaps the mm1 matmuls of batches b+1/b+2; the final reduce accumulates all
8 batches into one [8,512] PSUM tile via per-batch one-hot lhsT columns.

Sharding: the 4096 sample points are split 512-per-core across 8 NeuronCores
(data-parallel in n); every core gets the full activations. Host concatenates
the per-core [8,512] outputs. No collectives needed.
"""

import os
import sys

import numpy as np

if "/opt/trn_rl_repo" not in sys.path:
    sys.path.insert(0, "/opt/trn_rl_repo")

B, H, W = 8, 128, 128
N_TOTAL = 4096
N_CORES = 8
NS = N_TOTAL // N_CORES  # 512 samples per core
NCH = NS // 128          # n-chunks per core (4)

LAST_EXEC_TIME_NS = None

_CACHE = {}


def _build_bass():
    import concourse.mybir as mybir
    import concourse.tile as tile
    from concourse import bacc

    f32 = mybir.dt.float32
    f32r = mybir.dt.float32r
    f16 = mybir.dt.float16

    nc = bacc.Bacc()

    acts_d = nc.declare_dram_parameter("activations", [B, H, W], f32, isOutput=False)
    # bundle columns: [mu (t,c): 8 | sigma (t,c): 8 | identity: 128]
    bund_d = nc.declare_dram_parameter("bundle", [128, 144], f32, isOutput=False)
    # zconst rows: {ones(H), -grid(H)}
    zconst_d = nc.declare_dram_parameter("zconst", [2, H], f32r, isOutput=False)
    oneh_d = nc.declare_dram_parameter("onehots", [W, 4 * 4], f16, isOutput=False)
    out_d = nc.declare_dram_parameter("out", [B, NS], f32, isOutput=True)

    # Derivative_Erf(x) = (2/sqrt(pi)) * exp(-x^2); with input scale 1/sqrt(2)
    # it yields c*exp(-0.5 z^2), c = 2/sqrt(pi). The c^2 from the two tables
    # and the 1/(H*W) mean fold into the final output scale.
    DErf = mybir.ActivationFunctionType.Derivative_Erf
    INV_SQRT2 = 0.7071067811865476
    OUT_SCALE = float(np.pi / (4.0 * H * W))

    with tile.TileContext(nc) as tc, nc.allow_low_precision(
        reason="float32r/f16 matmul inputs are intentional"
    ):
        with (
            tc.tile_pool(name="const", bufs=1) as constp,
            tc.tile_pool(name="io", bufs=1) as iop,
            tc.tile_pool(name="vbuf", bufs=4) as vp,
            tc.tile_pool(name="psz", bufs=2, space="PSUM") as pszp,
            tc.tile_pool(name="psn", bufs=4, space="PSUM") as psnp,
            tc.tile_pool(name="pso", bufs=2, space="PSUM") as psop,
        ):
            # ---- loads: bundle on sync ring, consts on scalar, acts on gpsimd ----
            bund = constp.tile([128, 144], f32)
            nc.sync.dma_start(bund[:], bund_d[:])
            mu_cols = bund[:, 0:8].rearrange("p (t c) -> p t c", c=NCH)
            sig_cols = bund[:, 8:16].rearrange("p (t c) -> p t c", c=NCH)
            ident = bund[:, 16:144]

            zconst = constp.tile([2, H], f32r)
            nc.scalar.dma_start(zconst[:], zconst_d[:])

            acts_sb = iop.tile([H, B, W], f16)
            nc.gpsimd.dma_start(acts_sb[:], acts_d[:].rearrange("b h w -> h b w"))
            oneh = constp.tile([W, 4 * 4], f16)
            nc.gpsimd.dma_start(oneh[:], oneh_d[:])

            # ---- prep columns [128, (q, t, c)], q in {mu/sigma, 1/sigma} ----
            cols = iop.tile([128, 2, 2, NCH], f32)
            nc.vector.reciprocal(cols[:, 1, :, :], sig_cols)
            nc.vector.tensor_mul(cols[:, 0, :, :], mu_cols, cols[:, 1, :, :])

            # transpose all 16 columns at once -> [16, 128] rows
            # (shares the z-table PSUM slots; released before the z matmuls)
            tps = pszp.tile([2 * 2 * NCH, 128], f32, tag="z", name="tps")
            nc.tensor.transpose(
                tps[:], cols[:].rearrange("p q t c -> p (q t c)"), ident
            )
            tsb = iop.tile([2 * 2 * NCH, 128], f32r)
            nc.vector.tensor_copy(tsb[:], tps[:])

            # PE warmups: keep the tensor engine continuously busy from the
            # transpose through the z matmuls so HAM is at full clock for the
            # batch loop. They rotate through the z-tag PSUM slots, which
            # chains them after the transpose without delaying the z matmuls.
            warm_rhs = acts_sb[:].rearrange("h b w -> h (b w)")[:, 0:512]
            for wi in range(5):
                ps_warm = pszp.tile(
                    [128, 512], f32, tag="z", name=f"ps_warm{wi}"
                )
                nc.tensor.matmul(
                    ps_warm[:], lhsT=acts_sb[:, 0, :], rhs=warm_rhs,
                    start=True, stop=True,
                )

            # gather one [2, NS] row tile per axis: rows {mu/sigma, 1/sigma};
            # one DMA per (q, t) — row q of zr gets tsb rows (q, t, 0..3)
            zrows = []
            for t in range(2):
                zr = iop.tile([2, NS], f32r, tag=f"zr{t}", name=f"zr{t}")
                for q in range(2):
                    j = (q * 2 + t) * NCH
                    eng = nc.sync if t == 0 else nc.scalar
                    eng.dma_start(
                        zr[q:q + 1, :].rearrange("one (c p) -> one c p", c=NCH),
                        tsb[j:j + NCH, :],
                    )
                zrows.append(zr)

            # ---- z = K=2 matmul (f32r); one Derivative_Erf per table ----
            def make_table(t, ptab_tile):
                ps_z = pszp.tile([H, NS], f32, tag="z", name=f"ps_z{t}")
                nc.tensor.matmul(
                    ps_z[:], lhsT=zconst[:], rhs=zrows[t][:], start=True, stop=True
                )
                nc.scalar.activation(ptab_tile[:], ps_z[:], DErf, scale=INV_SQRT2)

            Ph = iop.tile([H, NS], f16)
            Pw = iop.tile([W, NS], f32)

            # ---- batch loop: groups of 4, column-tiled concurrent reduces ----
            ps_n = [None] * B
            vs = [None] * B

            def mm1(b):
                ps_n[b] = psnp.tile([W, NS], f32, tag="n", name=f"ps_n{b}")
                nc.tensor.matmul(
                    ps_n[b][:], lhsT=acts_sb[:, b, :], rhs=Ph[:],
                    start=True, stop=True,
                )

            def vmul(b):
                vs[b] = vp.tile([W, NS], f16, tag="v", name=f"v{b}")
                nc.vector.tensor_mul(vs[b][:], ps_n[b][:], Pw[:])

            make_table(0, Ph)   # Ph first: gates the mm1 stream
            for b in range(4):
                mm1(b)
            make_table(1, Pw)   # Pw only gates the DVE multiplies
            for b in range(4):
                vmul(b)
            for b in range(4, B):
                mm1(b)
            for b in range(4, B):
                vmul(b)

            ps_o = [None, None]
            for g in range(2):
                # accumulate 4 batches into rows 0-3 via one-hot lhsT columns
                ps_o[g] = psop.tile([4, NS], f32, tag="o", name=f"ps_o{g}")
                for k in range(4):
                    nc.tensor.matmul(
                        ps_o[g][:], lhsT=oneh[:, k * 4:(k + 1) * 4],
                        rhs=vs[4 * g + k][:], start=(k == 0), stop=(k == 3),
                    )
                rsb = iop.tile([4, NS], f32, tag=f"r{g}", name=f"rsb{g}")
                nc.vector.tensor_scalar_mul(rsb[:], ps_o[g][:], OUT_SCALE)
                eng = nc.sync if g == 0 else nc.scalar
                eng.dma_start(out_d[g * 4:(g + 1) * 4, :], rsb[:])

    nc.compile()
    return nc


def _constants():
    gh = np.arange(H, dtype=np.float32) / H
    zconst = np.ascontiguousarray(
        np.stack([np.ones(H, np.float32), -gh]).astype(np.float32)
    )
    oneh = np.zeros((W, 4 * 4), np.float16)
    for j in range(4):
        oneh[:, j * 4 + j] = 1.0
    ident = np.eye(128, dtype=np.float32)
    return zconst, oneh, ident


def _bundle(mu_sl, sig_sl, ident):
    # [128, 8 | 8 | 128]: mu/sigma in (t, c) column order, then identity
    mu_cols = mu_sl.reshape(NCH, 128, 2).transpose(1, 2, 0).reshape(128, 8)
    sig_cols = sig_sl.reshape(NCH, 128, 2).transpose(1, 2, 0).reshape(128, 8)
    return np.ascontiguousarray(
        np.concatenate([mu_cols, sig_cols, ident], axis=1).astype(np.float32)
    )


def kernel(activations, mu, sigma):
    from concourse.bass_utils import run_bass_kernel_spmd

    global LAST_EXEC_TIME_NS

    activations = np.ascontiguousarray(np.asarray(activations, dtype=np.float32))
    mu = np.ascontiguousarray(np.asarray(mu, dtype=np.float32))
    sigma = np.ascontiguousarray(np.asarray(sigma, dtype=np.float32))
    assert activations.shape == (B, H, W)
    assert mu.shape == (N_TOTAL, 2) and sigma.shape == (N_TOTAL, 2)

    if "nc" not in _CACHE:
        _CACHE["nc"] = _build_bass()
    nc = _CACHE["nc"]

    zconst, oneh, ident = _constants()
    in_maps = []
    for c in range(N_CORES):
        sl = slice(c * NS, (c + 1) * NS)
        in_maps.append(
            {
                "activations": activations,
                "bundle": _bundle(mu[sl], sigma[sl], ident),
                "zconst": zconst,
                "onehots": oneh,
            }
        )

    res = run_bass_kernel_spmd(nc, in_maps, core_ids=list(range(N_CORES)))
    LAST_EXEC_TIME_NS = res.exec_time_ns

    out = np.concatenate([r["out"] for r in res.results], axis=1)  # [B, N_TOTAL]
    return out.reshape(B, 64, 64).astype(np.float32)


# revision 43
# speedup vs baseline: 1.1694x; 1.1694x over previous
"""Trainium2 Bass kernel for AnisotropicGaussianSampler.

Reference computation (H=W=128, N=4096 samples, B=8):
    corr[b,n] = (1/(H*W)) * sum_{h,w} A[b,h,w] * exp(-eh[h,n]) * exp(-ew[w,n])
    eh[h,n] = (h/H - mu[n,0])^2 / (2*sigma[n,0]^2)   (separable in h and w)

Factorization used on-device (per sample column n):
    Ph[h,n] = exp(-0.5 * zh^2),  zh = (mu0[n] - h/H) / sigma0[n]
    Pw[w,n] = exp(-0.5 * zw^2)
    N_b[w,n] = sum_h A[b,h,w] * Ph[h,n]          (matmul, lhsT = A_b as stored)
    corr[b,n] = (1/(H*W)) * sum_w Pw[w,n]*N_b[w,n]  (mul + ones-reduce matmul)

Precision split: the z tables are produced in float32r (single-pass fp32
matmul; z is cancellation-sensitive), while the big batch matmuls run in
float16 (same 1 cycle/row as f32r but ~10x faster weight loads via FWL;
fp16's 11-bit mantissa keeps the result within ~2e-3).

Table prep: 1/sigma and mu/sigma are computed across 128 partitions (fast DVE)
in a [128, (q,t,c)] column tile, PE-transposed to [16, 128] in one shot, copied
to SBUF (rounding to f32r), and DMA-gathered into one [2, 512] row tile per
axis. A single K=2 matmul per axis (constant lhsT rows {ones, -grid}) then
produces z in PSUM; ACT squares and exponentiates it.

DMA routing: all small loads are packed into ONE [128, 144] bundle (mu, sigma,
identity) on the sync HWDGE ring; zconst/onehots ride the scalar ring; the
512KB activations load is a single casting DMA (f32 -> f16) on gpsimd SWDGE.

The batch loop is software-pipelined (skew 2) so the DVE multiply of batch b
overlaps the mm1 matmuls of batches b+1/b+2; the final reduce accumulates all
8 batches into one [8,512] PSUM tile via per-batch one-hot lhsT columns.

Sharding: the 4096 sample points are split 512-per-core across 8 NeuronCores
(data-parallel in n); every core gets the full activations. Host concatenates
the per-core [8,512] outputs. No collectives needed.
"""

import os
import sys

import numpy as np

if "/opt/trn_rl_repo" not in sys.path:
    sys.path.insert(0, "/opt/trn_rl_repo")

B, H, W = 8, 128, 128
N_TOTAL = 4096
N_CORES = 8
NS = N_TOTAL // N_CORES  # 512 samples per core
NCH = NS // 128          # n-chunks per core (4)

LAST_EXEC_TIME_NS = None

_CACHE = {}


def _build_bass():
    import concourse.mybir as mybir
    import concourse.tile as tile
    from concourse import bacc

    f32 = mybir.dt.float32
    f32r = mybir.dt.float32r
    f16 = mybir.dt.float16

    nc = bacc.Bacc()

    acts_d = nc.declare_dram_parameter("activations", [B, H, W], f32, isOutput=False)
    # bundle columns: [mu (t,c): 8 | sigma (t,c): 8 | identity: 128]
    bund_d = nc.declare_dram_parameter("bundle", [128, 144], f32, isOutput=False)
    # zconst rows: {ones(H), -grid(H)}
    zconst_d = nc.declare_dram_parameter("zconst", [2, H], f32r, isOutput=False)
    oneh_d = nc.declare_dram_parameter("onehots", [W, 4 * 4], f16, isOutput=False)
    out_d = nc.declare_dram_parameter("out", [B, NS], f32, isOutput=True)

    # Derivative_Erf(x) = (2/sqrt(pi)) * exp(-x^2); with input scale 1/sqrt(2)
    # it yields c*exp(-0.5 z^2), c = 2/sqrt(pi). The c^2 from the two tables
    # and the 1/(H*W) mean fold into the final output scale.
    DErf = mybir.ActivationFunctionType.Derivative_Erf
    INV_SQRT2 = 0.7071067811865476
    OUT_SCALE = float(np.pi / (4.0 * H * W))

    with tile.TileContext(nc) as tc, nc.allow_low_precision(
        reason="float32r/f16 matmul inputs are intentional"
    ):
        with (
            tc.tile_pool(name="const", bufs=1) as constp,
            tc.tile_pool(name="io", bufs=1) as iop,
            tc.tile_pool(name="vbuf", bufs=4) as vp,
            tc.tile_pool(name="psz", bufs=2, space="PSUM") as pszp,
            tc.tile_pool(name="psn", bufs=4, space="PSUM") as psnp,
            tc.tile_pool(name="pso", bufs=2, space="PSUM") as psop,
        ):
            # ---- loads: bundle on sync ring, consts on scalar, acts on gpsimd ----
            bund = constp.tile([128, 144], f32)
            nc.sync.dma_start(bund[:], bund_d[:])
            mu_cols = bund[:, 0:8].rearrange("p (t c) -> p t c", c=NCH)
            sig_cols = bund[:, 8:16].rearrange("p (t c) -> p t c", c=NCH)
            ident = bund[:, 16:144]

            zconst = constp.tile([2, H], f32r)
            nc.scalar.dma_start(zconst[:], zconst_d[:])

            # dummy activation issued first so the Derivative_Erf function
            # table loads during the DMA phase, not on the table critical path
            dummy = constp.tile([1, 1], f32, name="dummy")
            nc.scalar.activation(
                dummy[:], nc.const_aps.tensor(1.0, (1, 1)), DErf, scale=1.0
            )

            acts_sb = iop.tile([H, B, W], f16)
            nc.gpsimd.dma_start(acts_sb[:], acts_d[:].rearrange("b h w -> h b w"))
            oneh = constp.tile([W, 4 * 4], f16)
            nc.gpsimd.dma_start(oneh[:], oneh_d[:])

            # ---- prep columns [128, (q, t, c)], q in {mu/sigma, 1/sigma} ----
            cols = iop.tile([128, 2, 2, NCH], f32)
            nc.vector.reciprocal(cols[:, 1, :, :], sig_cols)
            nc.vector.tensor_mul(cols[:, 0, :, :], mu_cols, cols[:, 1, :, :])

            # transpose all 16 columns at once -> [16, 128] rows
            # (shares the z-table PSUM slots; released before the z matmuls)
            tps = pszp.tile([2 * 2 * NCH, 128], f32, tag="z", name="tps")
            nc.tensor.transpose(
                tps[:], cols[:].rearrange("p q t c -> p (q t c)"), ident
            )
            tsb = iop.tile([2 * 2 * NCH, 128], f32r)
            nc.vector.tensor_copy(tsb[:], tps[:])

            # gather one [2, NS] row tile per axis: rows {mu/sigma, 1/sigma};
            # one DMA per (q, t) — row q of zr gets tsb rows (q, t, 0..3)
            zrows = []
            for t in range(2):
                zr = iop.tile([2, NS], f32r, tag=f"zr{t}", name=f"zr{t}")
                for q in range(2):
                    j = (q * 2 + t) * NCH
                    eng = nc.sync if t == 0 else nc.scalar
                    eng.dma_start(
                        zr[q:q + 1, :].rearrange("one (c p) -> one c p", c=NCH),
                        tsb[j:j + NCH, :],
                    )
                zrows.append(zr)

            # ---- z = K=2 matmul (f32r); one Derivative_Erf per table ----
            def make_table(t, ptab_tile):
                ps_z = pszp.tile([H, NS], f32, tag="z", name=f"ps_z{t}")
                nc.tensor.matmul(
                    ps_z[:], lhsT=zconst[:], rhs=zrows[t][:], start=True, stop=True
                )
                nc.scalar.activation(ptab_tile[:], ps_z[:], DErf, scale=INV_SQRT2)

            Ph = iop.tile([H, NS], f16)
            Pw = iop.tile([W, NS], f32)

            # ---- batch loop: groups of 4, column-tiled concurrent reduces ----
            ps_n = [None] * B
            vs = [None] * B

            def mm1(b):
                ps_n[b] = psnp.tile([W, NS], f32, tag="n", name=f"ps_n{b}")
                nc.tensor.matmul(
                    ps_n[b][:], lhsT=acts_sb[:, b, :], rhs=Ph[:],
                    start=True, stop=True,
                )

            def vmul(b):
                vs[b] = vp.tile([W, NS], f16, tag="v", name=f"v{b}")
                nc.vector.tensor_mul(vs[b][:], ps_n[b][:], Pw[:])

            make_table(0, Ph)   # Ph first: gates the mm1 stream
            for b in range(4):
                mm1(b)
            make_table(1, Pw)   # Pw only gates the DVE multiplies
            for b in range(4):
                vmul(b)
            for b in range(4, B):
                mm1(b)
            for b in range(4, B):
                vmul(b)

            ps_o = [None, None]
            for g in range(2):
                # accumulate 4 batches into rows 0-3 via one-hot lhsT columns
                ps_o[g] = psop.tile([4, NS], f32, tag="o", name=f"ps_o{g}")
                for k in range(4):
                    nc.tensor.matmul(
                        ps_o[g][:], lhsT=oneh[:, k * 4:(k + 1) * 4],
                        rhs=vs[4 * g + k][:], start=(k == 0), stop=(k == 3),
                    )
                rsb = iop.tile([4, NS], f32, tag=f"r{g}", name=f"rsb{g}")
                nc.vector.tensor_scalar_mul(rsb[:], ps_o[g][:], OUT_SCALE)
                eng = nc.sync if g == 0 else nc.scalar
                eng.dma_start(out_d[g * 4:(g + 1) * 4, :], rsb[:])

    nc.compile()
    return nc


def _constants():
    gh = np.arange(H, dtype=np.float32) / H
    zconst = np.ascontiguousarray(
        np.stack([np.ones(H, np.float32), -gh]).astype(np.float32)
    )
    oneh = np.zeros((W, 4 * 4), np.float16)
    for j in range(4):
        oneh[:, j * 4 + j] = 1.0
    ident = np.eye(128, dtype=np.float32)
    return zconst, oneh, ident


def _bundle(mu_sl, sig_sl, ident):
    # [128, 8 | 8 | 128]: mu/sigma in (t, c) column order, then identity
    mu_cols = mu_sl.reshape(NCH, 128, 2).transpose(1, 2, 0).reshape(128, 8)
    sig_cols = sig_sl.reshape(NCH, 128, 2).transpose(1, 2, 0).reshape(128, 8)
    return np.ascontiguousarray(
        np.concatenate([mu_cols, sig_cols, ident], axis=1).astype(np.float32)
    )


def kernel(activations, mu, sigma):
    from concourse.bass_utils import run_bass_kernel_spmd

    global LAST_EXEC_TIME_NS

    activations = np.ascontiguousarray(np.asarray(activations, dtype=np.float32))
    mu = np.ascontiguousarray(np.asarray(mu, dtype=np.float32))
    sigma = np.ascontiguousarray(np.asarray(sigma, dtype=np.float32))
    assert activations.shape == (B, H, W)
    assert mu.shape == (N_TOTAL, 2) and sigma.shape == (N_TOTAL, 2)

    if "nc" not in _CACHE:
        _CACHE["nc"] = _build_bass()
    nc = _CACHE["nc"]

    zconst, oneh, ident = _constants()
    in_maps = []
    for c in range(N_CORES):
        sl = slice(c * NS, (c + 1) * NS)
        in_maps.append(
            {
                "activations": activations,
                "bundle": _bundle(mu[sl], sigma[sl], ident),
                "zconst": zconst,
                "onehots": oneh,
            }
        )

    res = run_bass_kernel_spmd(nc, in_maps, core_ids=list(range(N_CORES)))
    LAST_EXEC_TIME_NS = res.exec_time_ns

    out = np.concatenate([r["out"] for r in res.results], axis=1)  # [B, N_TOTAL]
    return out.reshape(B, 64, 64).astype(np.float32)
